# revision 15
# baseline (speedup 1.0000x reference)
"""Trainium2 Bass kernel for the 2-graph GCN (nn_Net_39041252721058).

Strategy (8 NeuronCores, SPMD single program):
  - Core k owns dst nodes [k*6250, (k+1)*6250). All edges with dst in that
    range are processed by core k, grouped by 128-node dst blocks.
  - Layer math uses the linearity of spmm: project first (x@W1 row-sharded,
    AllGather of the projected bf16 table), then per-edge gather rows of the
    table with SWDGE dma_gather (int16 indices; src split in two classes at
    row 32768 so indices fit), then segment-sum via one-hot matmuls on the
    tensor engine accumulating in PSUM (edges on the contraction axis).
  - The one-hot dst-slot selection matrices (sel = onehot(slot) * w) are
    precomputed on the HOST and streamed from DRAM per block (they are
    identical for layers 1 and 2, so each graph's 54 MB sel array is read
    twice). This removes the vector-engine is_equal/mult builds (~1.9 ms)
    from the critical path at the cost of ~110 MB/core extra DMA reads,
    which ride on otherwise-idle HWDGE/SDMA bandwidth.
  - h = relu(agg + b1) stays on-chip per block; support2 = h @ W2 uses PE
    transposes of h; AllGather of support2, then the same gather/scatter
    machinery for layer 2.
  - The two graphs' phases are interleaved (A_d, AG1_d, A_s, AG1_s, B_d,
    AG2_d, B_s, AG2_s, C_d, C_s) so all collectives except the first hide
    behind the other graph's gather stream.
  - Phase A interleaves two PSUM accumulators (even/odd k-tiles) so PE
    weight loads pipeline; the halves are combined via a scalar-engine
    PSUM->SBUF copy plus a vector add (a vector op may read only one PSUM
    operand).

Perf model (measured on trn2): the kernel is bound by the SWDGE gather
ucode, ~135 ns per 16-row descriptor batch (~8.4 ns/row) per Q7 core
pair, insensitive to row bytes and to index order. The descriptor
generation is parallelized across the 4 SWDGE queues (queue q runs on Q7
cores 2q, 2q+1; each queue has its own full-size descriptor ring in its
own 32-partition group), assigned round-robin per gather call. This took
the gather stream from 6.7 ms serial to ~2x-3x concurrent. Gather calls
are batched to 1536 rows (96 ring entries; >=184 crashes the ucode), use
per-BLOCK gather counts (ceil-16 of the max count over the 8 cores), and
single_packet=False (True crashes the ucode at 1536-row calls).

Correctness details: pads carry sel=0 so the one-hot multiply zeroes
them; message buffers are memset once on first use so never-gathered pad
slots cannot inject NaN (0*NaN = NaN would poison PSUM).
"""
import numpy as np
import ml_dtypes

import concourse.bass as bass
import concourse.bacc as bacc
import concourse.mybir as mybir
import concourse.tile as tile
from concourse.bass_utils import run_bass_kernel_spmd

NCORES = 8
NODES = 50000
PER_CORE = NODES // NCORES           # 6250
NBLK = (PER_CORE + 127) // 128       # 49 (last block has 106 nodes)
SPLIT = 32768                        # int16 gather-index class boundary
F_IN = 512
H1 = 256
H2 = 128

BF16 = ml_dtypes.bfloat16


# ----------------------------------------------------------------------------
# Host-side edge preprocessing
# ----------------------------------------------------------------------------

def _analyze(src, dst):
    core = dst // PER_CORE
    blk = (dst % PER_CORE) // 128
    cls = (src >= SPLIT).astype(np.int64)
    key = (core * NBLK + blk) * 2 + cls
    counts = np.bincount(key, minlength=NCORES * NBLK * 2).reshape(-1, 2)
    return int(counts[:, 0].max()), int(counts[:, 1].max())


def _analyze_blocks(src, dst):
    """Per-block max-over-cores counts, ceil-16, per class: [NBLK, 2] int."""
    core = dst // PER_CORE
    blk = (dst % PER_CORE) // 128
    cls = (src >= SPLIT).astype(np.int64)
    key = (core * NBLK + blk) * 2 + cls
    counts = np.bincount(key, minlength=NCORES * NBLK * 2)
    counts = counts.reshape(NCORES, NBLK, 2).max(axis=0)
    return np.maximum(16, -(-counts // 16) * 16)


def _sel_layout(N16):
    """Per-block sel tile counts and column offsets (shared across cores).
    Returns (TA[NBLK], TB[NBLK], coloff[NBLK], total_cols)."""
    TA = -(-N16[:, 0] // 128)
    TB = -(-N16[:, 1] // 128)
    T = TA + TB
    coloff = np.zeros(NBLK, np.int64)
    coloff[1:] = np.cumsum(T[:-1]) * 128
    return TA, TB, coloff, int(T.sum() * 128)


def _prep_graph(src, dst, w, G_A, G_B, N16, calls, qcols):
    """Per-core gather index arrays and host-built one-hot sel arrays."""
    TA, TB, coloff, selcols = _sel_layout(N16)
    core_all = dst // PER_CORE
    out = []
    for k in range(NCORES):
        m = core_all == k
        s, ww = src[m], w[m]
        rel = dst[m] - k * PER_CORE
        blk = rel // 128
        slot = rel % 128
        cls = (s >= SPLIT).astype(np.int64)
        order = np.lexsort((s, cls, blk))
        s, ww, blk, slot, cls = s[order], ww[order], blk[order], slot[order], cls[order]

        idxA = np.zeros((NBLK, G_A * 128), np.int16)
        idxB = np.zeros((NBLK, G_B * 128), np.int16)
        key = blk * 2 + cls
        cnt = np.bincount(key, minlength=NBLK * 2).reshape(NBLK, 2)
        assert cnt[:, 0].max() <= G_A * 128 and cnt[:, 1].max() <= G_B * 128
        starts = np.concatenate([[0], np.cumsum(cnt.ravel())])
        idxQ = np.zeros((128, qcols), np.int16)

        # position of each edge within its (block, class) run
        pos_in_run = np.arange(len(s)) - starts[key]
        # sel[partition, col] = w; col = coloff[blk] + (tile + TA[blk]*isB)*128 + slot
        tile_i = pos_in_run // 128
        part = pos_in_run % 128
        coltile = coloff[blk] // 128 + tile_i + np.where(cls == 1, TA[blk], 0)
        # compact packed (slot, w) arrays for the on-device DVE sel build
        pos_pk = np.zeros((128, selcols // 128), np.float32)
        wv_pk = np.zeros((128, selcols // 128), np.float32)
        pos_pk[part, coltile] = slot
        wv_pk[part, coltile] = ww

        for b in range(NBLK):
            nA, nB = cnt[b, 0], cnt[b, 1]
            oA, oB = starts[b * 2], starts[b * 2 + 1]
            idxA[b, :nA] = s[oA:oA + nA]
            idxB[b, :nB] = s[oB:oB + nB] - SPLIT

        # Pack each call's idx slice into its queue's 32-partition window:
        # queue q's Q7 pair (cores 2q, 2q+1) reads only partitions
        # [32q, 32q+32), so 4 queues' calls overlay in the same columns.
        for (b, cl, r0, n, q, qoff) in calls:
            a = idxA if cl == 0 else idxB
            sl = a[b, r0:r0 + n].reshape(n // 16, 16).T  # [16, n//16]
            idxQ[32 * q:32 * q + 16, qoff:qoff + n // 16] = sl
            idxQ[32 * q + 16:32 * q + 32, qoff:qoff + n // 16] = sl

        out.append({
            "idxQ": idxQ,
            "pos": pos_pk.astype(BF16),
            "wv": wv_pk.astype(BF16),
        })
    return out


def _prep_x(x, k):
    """Blocked transposed node features for core k: [NBLK*128, F_IN] bf16
    with row b*128+i, col kt*128+j = x[k*PER_CORE + b*128 + j, kt*128 + i]."""
    xs = np.zeros((NBLK * 128, F_IN), BF16)
    xk = x[k * PER_CORE:(k + 1) * PER_CORE].astype(BF16)  # [6250, 512]
    for b in range(NBLK):
        rows = min(128, PER_CORE - b * 128)
        blkT = xk[b * 128:b * 128 + rows].T  # [512, rows]
        t = blkT.reshape(4, 128, rows)       # [kt, i, j]
        xs[b * 128:(b + 1) * 128, :] = np.transpose(
            np.pad(t, ((0, 0), (0, 0), (0, 128 - rows))), (1, 0, 2)
        ).reshape(128, 512)
    return xs


# ----------------------------------------------------------------------------
# Device program
# ----------------------------------------------------------------------------

def _chunks16(total, mx=768):
    """Split `total` (multiple of 16) gather rows into calls of <=mx rows
    (mx=768 = 48 of the ring's ~128 16-row entries, so two calls fit in a
    queue's descriptor ring and the NX can dispatch one ahead)."""
    out, r0 = [], 0
    while r0 < total:
        n = min(mx, total - r0)
        out.append((r0, n))
        r0 += n
    return out


NQUEUES = 4


def _call_schedule(N16):
    """Static per-graph gather call list: [(b, cls, r0, n, q, off)].
    Least-loaded queue assignment (queue q = Q7 core pair 2q,2q+1) and
    per-queue packed idx column offsets. The same schedule serves phases B
    and C (identical call structure), so one idx tensor covers both.
    Returns (calls, QCOLS)."""
    load = [0] * NQUEUES
    off = [0] * NQUEUES
    calls = []
    for b in range(NBLK):
        for cls in (0, 1):
            for r0, n in _chunks16(int(N16[b, cls])):
                q = min(range(NQUEUES), key=lambda i: load[i])
                load[q] += n
                calls.append((b, cls, r0, n, q, off[q]))
                off[q] += n // 16
    return calls, max(off)


def _graph_setup(nc, tc, sb, ps, dr, p, G_A, G_B, tens, consts, N_A, N_B,
                 TA, TB, coloff, calls, qcols):
    """Load resident tiles + alloc DRAM intermediates for one graph."""
    GT = G_A + G_B
    dt = mybir.dt
    ident_t, ones_t, iota_t = consts

    # resident per-graph tiles
    w1_t = sb.tile([128, 4, H1], dt.bfloat16, tag="w1")
    nc.sync.dma_start(out=w1_t[:], in_=tens[p + "W1"][:].rearrange("(a b) c -> b a c", b=128))
    w2_t = sb.tile([128, 2, H2], dt.bfloat16, tag="w2")
    nc.sync.dma_start(out=w2_t[:], in_=tens[p + "W2"][:].rearrange("(a b) c -> b a c", b=128))
    b1_t = sb.tile([1, H1], dt.bfloat16, tag="b1")
    nc.sync.dma_start(out=b1_t[:], in_=tens[p + "b1"][:])
    b2_t = sb.tile([1, H2], dt.bfloat16, tag="b2")
    nc.sync.dma_start(out=b2_t[:], in_=tens[p + "b2"][:])
    idxQ_t = sb.tile([128, qcols], dt.int16, tag="idxQ")
    nc.sync.dma_start(out=idxQ_t[:], in_=tens[p + "idxQ"][:])
    ntiles = int((TA + TB).sum())
    pos_t = sb.tile([128, ntiles], dt.bfloat16, tag="pos")
    nc.sync.dma_start(out=pos_t[:], in_=tens[p + "pos"][:])
    wv_t = sb.tile([128, ntiles], dt.bfloat16, tag="wv")
    nc.sync.dma_start(out=wv_t[:], in_=tens[p + "wv"][:])

    # DRAM intermediates
    s1_own = dr.tile([PER_CORE, H1], dt.bfloat16, tag=p + "s1o")
    s1_full = dr.tile([NODES, H1], dt.bfloat16, tag=p + "s1f", addr_space="Shared")
    s2_own = dr.tile([PER_CORE, H2], dt.bfloat16, tag=p + "s2o")
    s2_full = dr.tile([NODES, H2], dt.bfloat16, tag=p + "s2f", addr_space="Shared")

    return dict(locals())


def _phase_A(st):
    nc, sb, ps, p, tens = st["nc"], st["sb"], st["ps"], st["p"], st["tens"]
    dt = mybir.dt
    w1_t, s1_own = st["w1_t"], st["s1_own"]
    # ---- Phase A: support1 = x @ W1 (own rows), 2 blocks per xt DMA ----
    for b0 in range(0, NBLK, 2):
        nb = min(2, NBLK - b0)
        xt = sb.tile([128, 2, 4, 128], dt.bfloat16, tag="xt", bufs=3)
        nc.sync.dma_start(
            out=xt[:, :nb, :, :],
            in_=tens[p + "xT"][b0 * 128:(b0 + nb) * 128, :]
                .rearrange("(t p) (a c) -> p t a c", p=128, a=4),
        )
        for t in range(nb):
            b = b0 + t
            rows = min(128, PER_CORE - b * 128)
            acc = ps.tile([128, H1], dt.float32, tag="acc256", bufs=3)
            for kt in range(4):
                nc.tensor.matmul(acc[:], lhsT=xt[:, t, kt, :], rhs=w1_t[:, kt, :],
                                 start=(kt == 0), stop=(kt == 3))
            s1sb = sb.tile([128, H1], dt.bfloat16, tag="s1sb", bufs=3)
            nc.vector.tensor_copy(out=s1sb[:], in_=acc[:])
            nc.sync.dma_start(out=s1_own[b * 128:b * 128 + rows, :], in_=s1sb[:rows, :])

    nc.gpsimd.collective_compute(
        "AllGather", mybir.AluOpType.bypass,
        replica_groups=[list(range(NCORES))],
        ins=[s1_own.opt()], outs=[st["s1_full"].opt()],
    )


def _phase_B(st):
    nc, sb, ps, p, tens = st["nc"], st["sb"], st["ps"], st["p"], st["tens"]
    dt = mybir.dt
    G_A, G_B, GT = st["G_A"], st["G_B"], st["GT"]
    N_A, N_B = st["N_A"], st["N_B"]
    TA, TB, coloff = st["TA"], st["TB"], st["coloff"]
    ident_t, ones_t, iota_t = st["ident_t"], st["ones_t"], st["iota_t"]
    idxQ_t = st["idxQ_t"]
    calls_by_block = st["calls_by_block"]
    pos_t, wv_t = st["pos_t"], st["wv_t"]
    w2_t, b1_t = st["w2_t"], st["b1_t"]
    s1_full, s2_own = st["s1_full"], st["s2_own"]

    # ---- Phase B: agg1 -> h -> support2 (own rows) ----
    # Software-pipelined: block b's relu runs on the scalar engine while the
    # PE streams block b+1's sel matmuls; the PE tail (transposes + W2) for
    # block b is issued after block b+1's sel matmuls so the PE never stalls
    # waiting on the scalar relu. Keeps the PE stream continuous (p-state).
    def tail(b, h_bf):
        rows = min(128, PER_CORE - b * 128)
        sp2 = ps.tile([128, H2], dt.float32, tag="acc128", bufs=2, name="sp2")
        tps = []
        for half in range(2):
            tp = ps.tile([128, 128], dt.bfloat16, tag="tp", name="tp")
            nc.tensor.transpose(out=tp[:], in_=h_bf[:, half * 128:(half + 1) * 128],
                                identity=ident_t[:])
            tps.append(tp)
        hTs = []
        for half in range(2):
            hT = sb.tile([128, 128], dt.bfloat16, tag="hT", bufs=4, name="hT")
            nc.vector.tensor_copy(out=hT[:], in_=tps[half][:])
            hTs.append(hT)
        for half in range(2):
            nc.tensor.matmul(sp2[:], lhsT=hTs[half][:], rhs=w2_t[:, half, :],
                             start=(half == 0), stop=(half == 1))
        s2sb = sb.tile([128, H2], dt.bfloat16, tag="s2sb", name="s2sb")
        nc.vector.tensor_copy(out=s2sb[:], in_=sp2[:])
        nc.sync.dma_start(out=s2_own[b * 128:b * 128 + rows, :], in_=s2sb[:rows, :])

    prev = None
    for b in range(NBLK):
        ta, tb = int(TA[b]), int(TB[b])
        msgsA = sb.tile([128, G_A, H1], dt.bfloat16, tag="mA", bufs=5)
        msgsB = sb.tile([128, G_B, H1], dt.bfloat16, tag="mB", bufs=5)
        if b < 5:
            nc.vector.memset(msgsA[:], 0.0)
            nc.vector.memset(msgsB[:], 0.0)
        for (cl, r0, n, q, qoff) in calls_by_block[b]:
            m, src_ap = (msgsA, s1_full[:]) if cl == 0 else (msgsB, s1_full[SPLIT:, :])
            g0, g1 = r0 // 128, (r0 + n + 127) // 128
            nc.gpsimd.dma_gather(
                m[:, g0:g1, :], src_ap,
                idxQ_t[:, qoff:qoff + n // 16],
                n, n, H1, single_packet=False, queue_num=q)

        # build sel = onehot(slot) * w on the vector engine (idle in B)
        toff = int(coloff[b]) // 128
        posb = pos_t[:, toff:toff + ta + tb]
        wvb = wv_t[:, toff:toff + ta + tb]
        ia = iota_t[:, :]
        iota_b = bass.AP(tensor=ia.tensor, offset=ia.offset,
                         ap=[ia.ap[0], [0, ta + tb], ia.ap[1]])
        selt = sb.tile([128, GT, 128], dt.bfloat16, tag="sel", bufs=3)
        nc.vector.tensor_tensor(out=selt[:, :ta + tb, :], in0=iota_b,
                                in1=posb.to_broadcast([128, ta + tb, 128]),
                                op=mybir.AluOpType.is_equal)
        nc.vector.tensor_tensor(out=selt[:, :ta + tb, :], in0=selt[:, :ta + tb, :],
                                in1=wvb.to_broadcast([128, ta + tb, 128]),
                                op=mybir.AluOpType.mult)

        acc = ps.tile([128, H1], dt.float32, tag="acc256", bufs=3)
        nc.tensor.matmul(acc[:], lhsT=ones_t[:], rhs=b1_t[:], start=True, stop=False)
        for c in range(ta):
            nc.tensor.matmul(acc[:], lhsT=selt[:, c, :], rhs=msgsA[:, c, :],
                             start=False, stop=False)
        for c in range(tb):
            nc.tensor.matmul(acc[:], lhsT=selt[:, ta + c, :], rhs=msgsB[:, c, :],
                             start=False, stop=(c == tb - 1))

        h_bf = sb.tile([128, H1], dt.bfloat16, tag="hbf", bufs=3)
        nc.scalar.activation(h_bf[:], acc[:], mybir.ActivationFunctionType.Relu)

        if prev is not None:
            tail(*prev)
        prev = (b, h_bf)
    tail(*prev)

    nc.gpsimd.collective_compute(
        "AllGather", mybir.AluOpType.bypass,
        replica_groups=[list(range(NCORES))],
        ins=[s2_own.opt()], outs=[st["s2_full"].opt()],
    )


def _phase_C(st):
    nc, sb, ps, p, tens = st["nc"], st["sb"], st["ps"], st["p"], st["tens"]
    dt = mybir.dt
    G_A, G_B, GT = st["G_A"], st["G_B"], st["GT"]
    N_A, N_B = st["N_A"], st["N_B"]
    TA, TB, coloff = st["TA"], st["TB"], st["coloff"]
    ones_t, iota_t = st["ones_t"], st["iota_t"]
    idxQ_t = st["idxQ_t"]
    calls_by_block = st["calls_by_block"]
    pos_t, wv_t = st["pos_t"], st["wv_t"]
    b2_t = st["b2_t"]
    s2_full = st["s2_full"]
    # ---- Phase C: agg2 + b2 -> out ----
    for b in range(NBLK):
        rows = min(128, PER_CORE - b * 128)
        ta, tb = int(TA[b]), int(TB[b])
        msgsA = sb.tile([128, G_A, H2], dt.bfloat16, tag="mA", bufs=5)
        msgsB = sb.tile([128, G_B, H2], dt.bfloat16, tag="mB", bufs=5)
        if b < 5:
            nc.vector.memset(msgsA[:], 0.0)
            nc.vector.memset(msgsB[:], 0.0)
        for (cl, r0, n, q, qoff) in calls_by_block[b]:
            m, src_ap = (msgsA, s2_full[:]) if cl == 0 else (msgsB, s2_full[SPLIT:, :])
            g0, g1 = r0 // 128, (r0 + n + 127) // 128
            nc.gpsimd.dma_gather(
                m[:, g0:g1, :], src_ap,
                idxQ_t[:, qoff:qoff + n // 16],
                n, n, H2, single_packet=False, queue_num=q)

        toff = int(coloff[b]) // 128
        posb = pos_t[:, toff:toff + ta + tb]
        wvb = wv_t[:, toff:toff + ta + tb]
        ia = iota_t[:, :]
        iota_b = bass.AP(tensor=ia.tensor, offset=ia.offset,
                         ap=[ia.ap[0], [0, ta + tb], ia.ap[1]])
        selt = sb.tile([128, GT, 128], dt.bfloat16, tag="sel", bufs=3)
        nc.vector.tensor_tensor(out=selt[:, :ta + tb, :], in0=iota_b,
                                in1=posb.to_broadcast([128, ta + tb, 128]),
                                op=mybir.AluOpType.is_equal)
        nc.vector.tensor_tensor(out=selt[:, :ta + tb, :], in0=selt[:, :ta + tb, :],
                                in1=wvb.to_broadcast([128, ta + tb, 128]),
                                op=mybir.AluOpType.mult)

        acc = ps.tile([128, H2], dt.float32, tag="acc128", bufs=2)
        nc.tensor.matmul(acc[:], lhsT=ones_t[:], rhs=b2_t[:], start=True, stop=False)
        for c in range(ta):
            nc.tensor.matmul(acc[:], lhsT=selt[:, c, :], rhs=msgsA[:, c, :],
                             start=False, stop=False)
        for c in range(tb):
            nc.tensor.matmul(acc[:], lhsT=selt[:, ta + c, :], rhs=msgsB[:, c, :],
                             start=False, stop=(c == tb - 1))

        ob = sb.tile([128, H2], dt.float32, tag="ob")
        nc.vector.tensor_copy(out=ob[:], in_=acc[:])
        nc.sync.dma_start(out=tens[p + "out"][b * 128:b * 128 + rows, :],
                          in_=ob[:rows, :])


def _build_program(GAd, GBd, GAs, GBs, N16):
    dt = mybir.dt
    nc = bacc.Bacc("TRN2", target_bir_lowering=False, debug=False,
                   num_devices=NCORES, num_swdge_queues=NQUEUES)
    tens = {}

    def inp(name, shape, dtype):
        tens[name] = nc.dram_tensor(name, shape, dtype, kind="ExternalInput")

    layouts = {}
    for p, (GA, GB) in (("d", (GAd, GBd)), ("s", (GAs, GBs))):
        layouts[p] = list(_sel_layout(N16[p]))
        inp(p + "xT", [NBLK * 128, F_IN], dt.bfloat16)
        inp(p + "W1", [F_IN, H1], dt.bfloat16)
        inp(p + "W2", [H1, H2], dt.bfloat16)
        inp(p + "b1", [1, H1], dt.bfloat16)
        inp(p + "b2", [1, H2], dt.bfloat16)
        calls, qcols = _call_schedule(N16[p])
        layouts[p] = layouts[p] + [calls, qcols]
        inp(p + "idxQ", [128, qcols], dt.int16)
        inp(p + "pos", [128, layouts[p][3] // 128], dt.bfloat16)
        inp(p + "wv", [128, layouts[p][3] // 128], dt.bfloat16)
        tens[p + "out"] = nc.dram_tensor(p + "out", [PER_CORE, H2], dt.float32,
                                         kind="ExternalOutput")
    inp("ident", [128, 128], dt.bfloat16)
    inp("ones", [1, 128], dt.bfloat16)
    inp("iota", [128, 128], dt.bfloat16)

    with tile.TileContext(nc) as tc:
        with (
            tc.tile_pool(name="sbuf", bufs=2) as sb,
            tc.tile_pool(name="psum", bufs=2, space="PSUM") as ps,
            tc.tile_pool(name="dram", bufs=1, space="DRAM") as dr,
        ):
            ident_t = sb.tile([128, 128], dt.bfloat16, tag="ident")
            nc.sync.dma_start(out=ident_t[:], in_=tens["ident"][:])
            ones_t = sb.tile([1, 128], dt.bfloat16, tag="ones")
            nc.sync.dma_start(out=ones_t[:], in_=tens["ones"][:])
            iota_t = sb.tile([128, 128], dt.bfloat16, tag="iota")
            nc.sync.dma_start(out=iota_t[:], in_=tens["iota"][:])
            consts = (ident_t, ones_t, iota_t)

            nd = N16["d"]
            ns_ = N16["s"]
            def mk(pp, GA, GB, nn):
                TA_, TB_, coloff_, _sc, calls_, qcols_ = layouts[pp]
                cbb = [[] for _ in range(NBLK)]
                for (b, cl, r0, n, q, qoff) in calls_:
                    cbb[b].append((cl, r0, n, q, qoff))
                st = _graph_setup(nc, tc, sb, ps, dr, pp, GA, GB, tens, consts,
                                  N_A=nn[:, 0], N_B=nn[:, 1],
                                  TA=TA_, TB=TB_, coloff=coloff_,
                                  calls=calls_, qcols=qcols_)
                st["calls_by_block"] = cbb
                return st
            std = mk("d", GAd, GBd, nd)
            sts = mk("s", GAs, GBs, ns_)
            _phase_A(std)
            _phase_A(sts)
            _phase_B(std)
            _phase_B(sts)
            _phase_C(std)
            _phase_C(sts)
    return nc


# ----------------------------------------------------------------------------
# Entry point
# ----------------------------------------------------------------------------

def kernel(drug_x, dis_x, drug_src, drug_dst, drug_w,
           dis_src, dis_dst, dis_w,
           W1d, b1d, W2d, b2d, W1s, b1s, W2s, b2s,
           _run_opts=None):
    graphs = {
        "d": (drug_x, drug_src, drug_dst, drug_w, W1d, b1d, W2d, b2d),
        "s": (dis_x, dis_src, dis_dst, dis_w, W1s, b1s, W2s, b2s),
    }
    G = {}
    N16 = {}
    preps = {}
    for p, (x, src, dst, w, W1, b1, W2, b2) in graphs.items():
        src = np.asarray(src); dst = np.asarray(dst); w = np.asarray(w)
        mA, mB = _analyze(src, dst)
        GA, GB = -(-mA // 128), -(-mB // 128)
        G[p] = (GA, GB)
        N16[p] = _analyze_blocks(src, dst)
        calls, qcols = _call_schedule(N16[p])
        preps[p] = _prep_graph(src, dst, w, GA, GB, N16[p], calls, qcols)

    nc = _build_program(G["d"][0], G["d"][1], G["s"][0], G["s"][1], N16)
    nc.compile()

    base = {
        "ident": np.eye(128, dtype=np.float32).astype(BF16),
        "ones": np.ones((1, 128), BF16),
        "iota": np.tile(np.arange(128, dtype=np.float32)[None, :].astype(BF16), (128, 1)),
    }
    for p, (x, src, dst, w, W1, b1, W2, b2) in graphs.items():
        base[p + "W1"] = np.asarray(W1).astype(BF16)
        base[p + "W2"] = np.asarray(W2).astype(BF16)
        base[p + "b1"] = np.asarray(b1).astype(BF16)[None, :]
        base[p + "b2"] = np.asarray(b2).astype(BF16)[None, :]

    in_maps = []
    for k in range(NCORES):
        m = dict(base)
        for p, (x, *_rest) in graphs.items():
            m[p + "xT"] = _prep_x(np.asarray(x), k)
            m.update({p + n: preps[p][k][n] for n in ("idxQ", "pos", "wv")})
        in_maps.append(m)

    res = run_bass_kernel_spmd(nc, in_maps, core_ids=list(range(NCORES)),
                               **(_run_opts or {}))
    emb1 = np.concatenate([res.results[k]["dout"] for k in range(NCORES)], axis=0)
    emb2 = np.concatenate([res.results[k]["sout"] for k in range(NCORES)], axis=0)
    if _run_opts:
        kernel.last_results = res
    return emb1, emb2


# revision 16
# speedup vs baseline: 1.0092x; 1.0092x over previous
"""Trainium2 Bass kernel for the 2-graph GCN (nn_Net_39041252721058).

Strategy (8 NeuronCores, SPMD single program):
  - Core k owns dst nodes [k*6250, (k+1)*6250). All edges with dst in that
    range are processed by core k, grouped by 128-node dst blocks.
  - Layer math uses the linearity of spmm: project first (x@W1 row-sharded,
    AllGather of the projected bf16 table), then per-edge gather rows of the
    table with SWDGE dma_gather (int16 indices; src split in two classes at
    row 32768 so indices fit), then segment-sum via one-hot matmuls on the
    tensor engine accumulating in PSUM (edges on the contraction axis).
  - The one-hot dst-slot selection matrices (sel = onehot(slot) * w) are
    precomputed on the HOST and streamed from DRAM per block (they are
    identical for layers 1 and 2, so each graph's 54 MB sel array is read
    twice). This removes the vector-engine is_equal/mult builds (~1.9 ms)
    from the critical path at the cost of ~110 MB/core extra DMA reads,
    which ride on otherwise-idle HWDGE/SDMA bandwidth.
  - h = relu(agg + b1) stays on-chip per block; support2 = h @ W2 uses PE
    transposes of h; AllGather of support2, then the same gather/scatter
    machinery for layer 2.
  - The two graphs' phases are interleaved (A_d, AG1_d, A_s, AG1_s, B_d,
    AG2_d, B_s, AG2_s, C_d, C_s) so all collectives except the first hide
    behind the other graph's gather stream.
  - Phase A interleaves two PSUM accumulators (even/odd k-tiles) so PE
    weight loads pipeline; the halves are combined via a scalar-engine
    PSUM->SBUF copy plus a vector add (a vector op may read only one PSUM
    operand).

Perf model (measured on trn2): the kernel is bound by the SWDGE gather
ucode, ~135 ns per 16-row descriptor batch (~8.4 ns/row) per Q7 core
pair, insensitive to row bytes and to index order. The descriptor
generation is parallelized across the 4 SWDGE queues (queue q runs on Q7
cores 2q, 2q+1; each queue has its own full-size descriptor ring in its
own 32-partition group), assigned round-robin per gather call. This took
the gather stream from 6.7 ms serial to ~2x-3x concurrent. Gather calls
are batched to 1536 rows (96 ring entries; >=184 crashes the ucode), use
per-BLOCK gather counts (ceil-16 of the max count over the 8 cores), and
single_packet=False (True crashes the ucode at 1536-row calls).

Correctness details: pads carry sel=0 so the one-hot multiply zeroes
them; message buffers are memset once on first use so never-gathered pad
slots cannot inject NaN (0*NaN = NaN would poison PSUM).
"""
import numpy as np
import ml_dtypes

import concourse.bass as bass
import concourse.bacc as bacc
import concourse.mybir as mybir
import concourse.tile as tile
from concourse.bass_utils import run_bass_kernel_spmd

NCORES = 8
NODES = 50000
PER_CORE = NODES // NCORES           # 6250
NBLK = (PER_CORE + 127) // 128       # 49 (last block has 106 nodes)
SPLIT = 32768                        # int16 gather-index class boundary
F_IN = 512
H1 = 256
H2 = 128

BF16 = ml_dtypes.bfloat16


# ----------------------------------------------------------------------------
# Host-side edge preprocessing
# ----------------------------------------------------------------------------

def _analyze(src, dst):
    core = dst // PER_CORE
    blk = (dst % PER_CORE) // 128
    cls = (src >= SPLIT).astype(np.int64)
    key = (core * NBLK + blk) * 2 + cls
    counts = np.bincount(key, minlength=NCORES * NBLK * 2).reshape(-1, 2)
    return int(counts[:, 0].max()), int(counts[:, 1].max())


def _analyze_blocks(src, dst):
    """Per-block max-over-cores counts, ceil-16, per class: [NBLK, 2] int."""
    core = dst // PER_CORE
    blk = (dst % PER_CORE) // 128
    cls = (src >= SPLIT).astype(np.int64)
    key = (core * NBLK + blk) * 2 + cls
    counts = np.bincount(key, minlength=NCORES * NBLK * 2)
    counts = counts.reshape(NCORES, NBLK, 2).max(axis=0)
    return np.maximum(16, -(-counts // 16) * 16)


def _sel_layout(N16):
    """Per-block sel tile counts and column offsets (shared across cores).
    Returns (TA[NBLK], TB[NBLK], coloff[NBLK], total_cols)."""
    TA = -(-N16[:, 0] // 128)
    TB = -(-N16[:, 1] // 128)
    T = TA + TB
    coloff = np.zeros(NBLK, np.int64)
    coloff[1:] = np.cumsum(T[:-1]) * 128
    return TA, TB, coloff, int(T.sum() * 128)


def _prep_graph(src, dst, w, G_A, G_B, N16, calls, qcols):
    """Per-core gather index arrays and host-built one-hot sel arrays."""
    TA, TB, coloff, selcols = _sel_layout(N16)
    core_all = dst // PER_CORE
    out = []
    for k in range(NCORES):
        m = core_all == k
        s, ww = src[m], w[m]
        rel = dst[m] - k * PER_CORE
        blk = rel // 128
        slot = rel % 128
        cls = (s >= SPLIT).astype(np.int64)
        order = np.lexsort((s, cls, blk))
        s, ww, blk, slot, cls = s[order], ww[order], blk[order], slot[order], cls[order]

        idxA = np.zeros((NBLK, G_A * 128), np.int16)
        idxB = np.zeros((NBLK, G_B * 128), np.int16)
        key = blk * 2 + cls
        cnt = np.bincount(key, minlength=NBLK * 2).reshape(NBLK, 2)
        assert cnt[:, 0].max() <= G_A * 128 and cnt[:, 1].max() <= G_B * 128
        starts = np.concatenate([[0], np.cumsum(cnt.ravel())])
        idxQ = np.zeros((128, qcols), np.int16)

        # position of each edge within its (block, class) run
        pos_in_run = np.arange(len(s)) - starts[key]
        # sel[partition, col] = w; col = coloff[blk] + (tile + TA[blk]*isB)*128 + slot
        tile_i = pos_in_run // 128
        part = pos_in_run % 128
        coltile = coloff[blk] // 128 + tile_i + np.where(cls == 1, TA[blk], 0)
        # compact packed (slot, w) arrays for the on-device DVE sel build
        pos_pk = np.zeros((128, selcols // 128), np.float32)
        wv_pk = np.zeros((128, selcols // 128), np.float32)
        pos_pk[part, coltile] = slot
        wv_pk[part, coltile] = ww

        for b in range(NBLK):
            nA, nB = cnt[b, 0], cnt[b, 1]
            oA, oB = starts[b * 2], starts[b * 2 + 1]
            idxA[b, :nA] = s[oA:oA + nA]
            idxB[b, :nB] = s[oB:oB + nB] - SPLIT

        # Pack each call's idx slice into its queue's 32-partition window:
        # queue q's Q7 pair (cores 2q, 2q+1) reads only partitions
        # [32q, 32q+32), so 4 queues' calls overlay in the same columns.
        for (b, cl, r0, n, q, qoff) in calls:
            a = idxA if cl == 0 else idxB
            sl = a[b, r0:r0 + n].reshape(n // 16, 16).T  # [16, n//16]
            idxQ[32 * q:32 * q + 16, qoff:qoff + n // 16] = sl
            idxQ[32 * q + 16:32 * q + 32, qoff:qoff + n // 16] = sl

        out.append({
            "idxQ": idxQ,
            "pos": pos_pk.astype(BF16),
            "wv": wv_pk.astype(BF16),
        })
    return out


def _prep_x(x, k):
    """Blocked transposed node features for core k: [NBLK*128, F_IN] bf16
    with row b*128+i, col kt*128+j = x[k*PER_CORE + b*128 + j, kt*128 + i]."""
    xs = np.zeros((NBLK * 128, F_IN), BF16)
    xk = x[k * PER_CORE:(k + 1) * PER_CORE].astype(BF16)  # [6250, 512]
    for b in range(NBLK):
        rows = min(128, PER_CORE - b * 128)
        blkT = xk[b * 128:b * 128 + rows].T  # [512, rows]
        t = blkT.reshape(4, 128, rows)       # [kt, i, j]
        xs[b * 128:(b + 1) * 128, :] = np.transpose(
            np.pad(t, ((0, 0), (0, 0), (0, 128 - rows))), (1, 0, 2)
        ).reshape(128, 512)
    return xs


# ----------------------------------------------------------------------------
# Device program
# ----------------------------------------------------------------------------

def _chunks16(total, mx=768):
    """Split `total` (multiple of 16) gather rows into calls of <=mx rows
    (mx=768 = 48 of the ring's ~128 16-row entries, so two calls fit in a
    queue's descriptor ring and the NX can dispatch one ahead)."""
    out, r0 = [], 0
    while r0 < total:
        n = min(mx, total - r0)
        out.append((r0, n))
        r0 += n
    return out


NQUEUES = 4


def _call_schedule(N16):
    """Static per-graph gather call list: [(b, cls, r0, n, q, off)].
    Least-loaded queue assignment (queue q = Q7 core pair 2q,2q+1) and
    per-queue packed idx column offsets. The same schedule serves phases B
    and C (identical call structure), so one idx tensor covers both.
    Returns (calls, QCOLS)."""
    load = [0] * NQUEUES
    off = [0] * NQUEUES
    calls = []
    for b in range(NBLK):
        for cls in (0, 1):
            for r0, n in _chunks16(int(N16[b, cls])):
                q = min(range(NQUEUES), key=lambda i: load[i])
                load[q] += n
                calls.append((b, cls, r0, n, q, off[q]))
                off[q] += n // 16
    return calls, max(off)


def _graph_setup(nc, tc, sb, ps, dr, p, G_A, G_B, tens, consts, N_A, N_B,
                 TA, TB, coloff, calls, qcols):
    """Load resident tiles + alloc DRAM intermediates for one graph."""
    GT = G_A + G_B
    dt = mybir.dt
    ident_t, ones_t, iota_t = consts

    # resident per-graph tiles
    w1_t = sb.tile([128, 4, H1], dt.bfloat16, tag="w1")
    nc.sync.dma_start(out=w1_t[:], in_=tens[p + "W1"][:].rearrange("(a b) c -> b a c", b=128))
    w2_t = sb.tile([128, 2, H2], dt.bfloat16, tag="w2")
    nc.sync.dma_start(out=w2_t[:], in_=tens[p + "W2"][:].rearrange("(a b) c -> b a c", b=128))
    b1_t = sb.tile([1, H1], dt.bfloat16, tag="b1")
    nc.sync.dma_start(out=b1_t[:], in_=tens[p + "b1"][:])
    b2_t = sb.tile([1, H2], dt.bfloat16, tag="b2")
    nc.sync.dma_start(out=b2_t[:], in_=tens[p + "b2"][:])
    idxQ_t = sb.tile([128, qcols], dt.int16, tag="idxQ")
    nc.sync.dma_start(out=idxQ_t[:], in_=tens[p + "idxQ"][:])
    ntiles = int((TA + TB).sum())
    pos_t = sb.tile([128, ntiles], dt.bfloat16, tag="pos")
    nc.sync.dma_start(out=pos_t[:], in_=tens[p + "pos"][:])
    wv_t = sb.tile([128, ntiles], dt.bfloat16, tag="wv")
    nc.sync.dma_start(out=wv_t[:], in_=tens[p + "wv"][:])

    # DRAM intermediates
    s1_own = dr.tile([PER_CORE, H1], dt.bfloat16, tag=p + "s1o")
    s1_full = dr.tile([NODES, H1], dt.bfloat16, tag=p + "s1f", addr_space="Shared")
    s2_own = dr.tile([PER_CORE, H2], dt.bfloat16, tag=p + "s2o")
    s2_full = dr.tile([NODES, H2], dt.bfloat16, tag=p + "s2f", addr_space="Shared")

    return dict(locals())


def _phase_A(st):
    nc, sb, ps, p, tens = st["nc"], st["sb"], st["ps"], st["p"], st["tens"]
    dt = mybir.dt
    w1_t, s1_own = st["w1_t"], st["s1_own"]
    # ---- Phase A: support1 = x @ W1 (own rows), 2 blocks per xt DMA ----
    for b0 in range(0, NBLK, 2):
        nb = min(2, NBLK - b0)
        xt = sb.tile([128, 2, 4, 128], dt.bfloat16, tag="xt", bufs=3)
        nc.sync.dma_start(
            out=xt[:, :nb, :, :],
            in_=tens[p + "xT"][b0 * 128:(b0 + nb) * 128, :]
                .rearrange("(t p) (a c) -> p t a c", p=128, a=4),
        )
        for t in range(nb):
            b = b0 + t
            rows = min(128, PER_CORE - b * 128)
            acc = ps.tile([128, H1], dt.float32, tag="acc256", bufs=3)
            for kt in range(4):
                nc.tensor.matmul(acc[:], lhsT=xt[:, t, kt, :], rhs=w1_t[:, kt, :],
                                 start=(kt == 0), stop=(kt == 3))
            s1sb = sb.tile([128, H1], dt.bfloat16, tag="s1sb", bufs=3)
            nc.vector.tensor_copy(out=s1sb[:], in_=acc[:])
            nc.sync.dma_start(out=s1_own[b * 128:b * 128 + rows, :], in_=s1sb[:rows, :])

    nc.gpsimd.collective_compute(
        "AllGather", mybir.AluOpType.bypass,
        replica_groups=[list(range(NCORES))],
        ins=[s1_own.opt()], outs=[st["s1_full"].opt()],
    )


def _phase_B(st):
    nc, sb, ps, p, tens = st["nc"], st["sb"], st["ps"], st["p"], st["tens"]
    dt = mybir.dt
    G_A, G_B, GT = st["G_A"], st["G_B"], st["GT"]
    N_A, N_B = st["N_A"], st["N_B"]
    TA, TB, coloff = st["TA"], st["TB"], st["coloff"]
    ident_t, ones_t, iota_t = st["ident_t"], st["ones_t"], st["iota_t"]
    idxQ_t = st["idxQ_t"]
    calls_by_block = st["calls_by_block"]
    nregs = st["nregs"]
    pos_t, wv_t = st["pos_t"], st["wv_t"]
    w2_t, b1_t = st["w2_t"], st["b1_t"]
    s1_full, s2_own = st["s1_full"], st["s2_own"]

    # ---- Phase B: agg1 -> h -> support2 (own rows) ----
    # Software-pipelined: block b's relu runs on the scalar engine while the
    # PE streams block b+1's sel matmuls; the PE tail (transposes + W2) for
    # block b is issued after block b+1's sel matmuls so the PE never stalls
    # waiting on the scalar relu. Keeps the PE stream continuous (p-state).
    def tail(b, h_bf):
        rows = min(128, PER_CORE - b * 128)
        sp2 = ps.tile([128, H2], dt.float32, tag="acc128", bufs=3, name="sp2")
        tps = []
        for half in range(2):
            tp = ps.tile([128, 128], dt.bfloat16, tag="tp", name="tp")
            nc.tensor.transpose(out=tp[:], in_=h_bf[:, half * 128:(half + 1) * 128],
                                identity=ident_t[:])
            tps.append(tp)
        hTs = []
        for half in range(2):
            hT = sb.tile([128, 128], dt.bfloat16, tag="hT", bufs=4, name="hT")
            nc.vector.tensor_copy(out=hT[:], in_=tps[half][:])
            hTs.append(hT)
        for half in range(2):
            nc.tensor.matmul(sp2[:], lhsT=hTs[half][:], rhs=w2_t[:, half, :],
                             start=(half == 0), stop=(half == 1))
        s2sb = sb.tile([128, H2], dt.bfloat16, tag="s2sb", name="s2sb")
        nc.vector.tensor_copy(out=s2sb[:], in_=sp2[:])
        nc.sync.dma_start(out=s2_own[b * 128:b * 128 + rows, :], in_=s2sb[:rows, :])

    prev = None
    for b in range(NBLK):
        ta, tb = int(TA[b]), int(TB[b])
        msgsA = sb.tile([128, G_A, H1], dt.bfloat16, tag="mA", bufs=5)
        msgsB = sb.tile([128, G_B, H1], dt.bfloat16, tag="mB", bufs=5)
        if b < 5:
            nc.vector.memset(msgsA[:], 0.0)
            nc.vector.memset(msgsB[:], 0.0)
        for (cl, r0, n, q, qoff) in calls_by_block[b]:
            m, src_ap = (msgsA, s1_full[:]) if cl == 0 else (msgsB, s1_full[SPLIT:, :])
            g0, g1 = r0 // 128, (r0 + n + 127) // 128
            nc.gpsimd.dma_gather(
                m[:, g0:g1, :], src_ap,
                idxQ_t[:, qoff:qoff + n // 16],
                n, nregs[n], H1, single_packet=False, queue_num=q)

        # build sel = onehot(slot) * w on the vector engine (idle in B)
        toff = int(coloff[b]) // 128
        posb = pos_t[:, toff:toff + ta + tb]
        wvb = wv_t[:, toff:toff + ta + tb]
        ia = iota_t[:, :]
        iota_b = bass.AP(tensor=ia.tensor, offset=ia.offset,
                         ap=[ia.ap[0], [0, ta + tb], ia.ap[1]])
        selt = sb.tile([128, GT, 128], dt.bfloat16, tag="sel", bufs=3)
        nc.vector.tensor_tensor(out=selt[:, :ta + tb, :], in0=iota_b,
                                in1=posb.to_broadcast([128, ta + tb, 128]),
                                op=mybir.AluOpType.is_equal)
        nc.vector.tensor_tensor(out=selt[:, :ta + tb, :], in0=selt[:, :ta + tb, :],
                                in1=wvb.to_broadcast([128, ta + tb, 128]),
                                op=mybir.AluOpType.mult)

        acc = ps.tile([128, H1], dt.float32, tag="acc256", bufs=3)
        nc.tensor.matmul(acc[:], lhsT=ones_t[:], rhs=b1_t[:], start=True, stop=False)
        for c in range(ta):
            nc.tensor.matmul(acc[:], lhsT=selt[:, c, :], rhs=msgsA[:, c, :],
                             start=False, stop=False)
        for c in range(tb):
            nc.tensor.matmul(acc[:], lhsT=selt[:, ta + c, :], rhs=msgsB[:, c, :],
                             start=False, stop=(c == tb - 1))

        h_bf = sb.tile([128, H1], dt.bfloat16, tag="hbf", bufs=3)
        nc.scalar.activation(h_bf[:], acc[:], mybir.ActivationFunctionType.Relu)

        if prev is not None:
            tail(*prev)
        prev = (b, h_bf)
    tail(*prev)

    nc.gpsimd.collective_compute(
        "AllGather", mybir.AluOpType.bypass,
        replica_groups=[list(range(NCORES))],
        ins=[s2_own.opt()], outs=[st["s2_full"].opt()],
    )


def _phase_C(st):
    nc, sb, ps, p, tens = st["nc"], st["sb"], st["ps"], st["p"], st["tens"]
    dt = mybir.dt
    G_A, G_B, GT = st["G_A"], st["G_B"], st["GT"]
    N_A, N_B = st["N_A"], st["N_B"]
    TA, TB, coloff = st["TA"], st["TB"], st["coloff"]
    ones_t, iota_t = st["ones_t"], st["iota_t"]
    idxQ_t = st["idxQ_t"]
    calls_by_block = st["calls_by_block"]
    nregs = st["nregs"]
    pos_t, wv_t = st["pos_t"], st["wv_t"]
    b2_t = st["b2_t"]
    s2_full = st["s2_full"]
    # ---- Phase C: agg2 + b2 -> out ----
    for b in range(NBLK):
        rows = min(128, PER_CORE - b * 128)
        ta, tb = int(TA[b]), int(TB[b])
        msgsA = sb.tile([128, G_A, H2], dt.bfloat16, tag="mA", bufs=5)
        msgsB = sb.tile([128, G_B, H2], dt.bfloat16, tag="mB", bufs=5)
        if b < 5:
            nc.vector.memset(msgsA[:], 0.0)
            nc.vector.memset(msgsB[:], 0.0)
        for (cl, r0, n, q, qoff) in calls_by_block[b]:
            m, src_ap = (msgsA, s2_full[:]) if cl == 0 else (msgsB, s2_full[SPLIT:, :])
            g0, g1 = r0 // 128, (r0 + n + 127) // 128
            nc.gpsimd.dma_gather(
                m[:, g0:g1, :], src_ap,
                idxQ_t[:, qoff:qoff + n // 16],
                n, nregs[n], H2, single_packet=False, queue_num=q)

        toff = int(coloff[b]) // 128
        posb = pos_t[:, toff:toff + ta + tb]
        wvb = wv_t[:, toff:toff + ta + tb]
        ia = iota_t[:, :]
        iota_b = bass.AP(tensor=ia.tensor, offset=ia.offset,
                         ap=[ia.ap[0], [0, ta + tb], ia.ap[1]])
        selt = sb.tile([128, GT, 128], dt.bfloat16, tag="sel", bufs=3)
        nc.vector.tensor_tensor(out=selt[:, :ta + tb, :], in0=iota_b,
                                in1=posb.to_broadcast([128, ta + tb, 128]),
                                op=mybir.AluOpType.is_equal)
        nc.vector.tensor_tensor(out=selt[:, :ta + tb, :], in0=selt[:, :ta + tb, :],
                                in1=wvb.to_broadcast([128, ta + tb, 128]),
                                op=mybir.AluOpType.mult)

        acc = ps.tile([128, H2], dt.float32, tag="acc128", bufs=3)
        nc.tensor.matmul(acc[:], lhsT=ones_t[:], rhs=b2_t[:], start=True, stop=False)
        for c in range(ta):
            nc.tensor.matmul(acc[:], lhsT=selt[:, c, :], rhs=msgsA[:, c, :],
                             start=False, stop=False)
        for c in range(tb):
            nc.tensor.matmul(acc[:], lhsT=selt[:, ta + c, :], rhs=msgsB[:, c, :],
                             start=False, stop=(c == tb - 1))

        ob = sb.tile([128, H2], dt.float32, tag="ob")
        nc.vector.tensor_copy(out=ob[:], in_=acc[:])
        nc.sync.dma_start(out=tens[p + "out"][b * 128:b * 128 + rows, :],
                          in_=ob[:rows, :])


def _build_program(GAd, GBd, GAs, GBs, N16):
    dt = mybir.dt
    nc = bacc.Bacc("TRN2", target_bir_lowering=False, debug=False,
                   num_devices=NCORES, num_swdge_queues=NQUEUES)
    tens = {}

    def inp(name, shape, dtype):
        tens[name] = nc.dram_tensor(name, shape, dtype, kind="ExternalInput")

    layouts = {}
    for p, (GA, GB) in (("d", (GAd, GBd)), ("s", (GAs, GBs))):
        layouts[p] = list(_sel_layout(N16[p]))
        inp(p + "xT", [NBLK * 128, F_IN], dt.bfloat16)
        inp(p + "W1", [F_IN, H1], dt.bfloat16)
        inp(p + "W2", [H1, H2], dt.bfloat16)
        inp(p + "b1", [1, H1], dt.bfloat16)
        inp(p + "b2", [1, H2], dt.bfloat16)
        calls, qcols = _call_schedule(N16[p])
        layouts[p] = layouts[p] + [calls, qcols]
        inp(p + "idxQ", [128, qcols], dt.int16)
        inp(p + "pos", [128, layouts[p][3] // 128], dt.bfloat16)
        inp(p + "wv", [128, layouts[p][3] // 128], dt.bfloat16)
        tens[p + "out"] = nc.dram_tensor(p + "out", [PER_CORE, H2], dt.float32,
                                         kind="ExternalOutput")
    inp("ident", [128, 128], dt.bfloat16)
    inp("ones", [1, 128], dt.bfloat16)
    inp("iota", [128, 128], dt.bfloat16)

    with tile.TileContext(nc) as tc:
        with (
            tc.tile_pool(name="sbuf", bufs=2) as sb,
            tc.tile_pool(name="psum", bufs=2, space="PSUM") as ps,
            tc.tile_pool(name="dram", bufs=1, space="DRAM") as dr,
        ):
            ident_t = sb.tile([128, 128], dt.bfloat16, tag="ident")
            nc.sync.dma_start(out=ident_t[:], in_=tens["ident"][:])
            ones_t = sb.tile([1, 128], dt.bfloat16, tag="ones")
            nc.sync.dma_start(out=ones_t[:], in_=tens["ones"][:])
            iota_t = sb.tile([128, 128], dt.bfloat16, tag="iota")
            nc.sync.dma_start(out=iota_t[:], in_=tens["iota"][:])
            consts = (ident_t, ones_t, iota_t)
            sizes = {n for pp in ("d", "s")
                     for (_b, _c, _r, n, _q, _o) in layouts[pp][4]}
            nregs = {n: nc.gpsimd.to_reg(n) for n in sorted(sizes)}

            nd = N16["d"]
            ns_ = N16["s"]
            def mk(pp, GA, GB, nn):
                TA_, TB_, coloff_, _sc, calls_, qcols_ = layouts[pp]
                cbb = [[] for _ in range(NBLK)]
                for (b, cl, r0, n, q, qoff) in calls_:
                    cbb[b].append((cl, r0, n, q, qoff))
                st = _graph_setup(nc, tc, sb, ps, dr, pp, GA, GB, tens, consts,
                                  N_A=nn[:, 0], N_B=nn[:, 1],
                                  TA=TA_, TB=TB_, coloff=coloff_,
                                  calls=calls_, qcols=qcols_)
                st["calls_by_block"] = cbb
                st["nregs"] = nregs
                return st
            std = mk("d", GAd, GBd, nd)
            sts = mk("s", GAs, GBs, ns_)
            _phase_A(std)
            _phase_A(sts)
            _phase_B(std)
            _phase_B(sts)
            _phase_C(std)
            _phase_C(sts)
    return nc


# ----------------------------------------------------------------------------
# Entry point
# ----------------------------------------------------------------------------

def kernel(drug_x, dis_x, drug_src, drug_dst, drug_w,
           dis_src, dis_dst, dis_w,
           W1d, b1d, W2d, b2d, W1s, b1s, W2s, b2s,
           _run_opts=None):
    graphs = {
        "d": (drug_x, drug_src, drug_dst, drug_w, W1d, b1d, W2d, b2d),
        "s": (dis_x, dis_src, dis_dst, dis_w, W1s, b1s, W2s, b2s),
    }
    G = {}
    N16 = {}
    preps = {}
    for p, (x, src, dst, w, W1, b1, W2, b2) in graphs.items():
        src = np.asarray(src); dst = np.asarray(dst); w = np.asarray(w)
        mA, mB = _analyze(src, dst)
        GA, GB = -(-mA // 128), -(-mB // 128)
        G[p] = (GA, GB)
        N16[p] = _analyze_blocks(src, dst)
        calls, qcols = _call_schedule(N16[p])
        preps[p] = _prep_graph(src, dst, w, GA, GB, N16[p], calls, qcols)

    nc = _build_program(G["d"][0], G["d"][1], G["s"][0], G["s"][1], N16)
    nc.compile()

    base = {
        "ident": np.eye(128, dtype=np.float32).astype(BF16),
        "ones": np.ones((1, 128), BF16),
        "iota": np.tile(np.arange(128, dtype=np.float32)[None, :].astype(BF16), (128, 1)),
    }
    for p, (x, src, dst, w, W1, b1, W2, b2) in graphs.items():
        base[p + "W1"] = np.asarray(W1).astype(BF16)
        base[p + "W2"] = np.asarray(W2).astype(BF16)
        base[p + "b1"] = np.asarray(b1).astype(BF16)[None, :]
        base[p + "b2"] = np.asarray(b2).astype(BF16)[None, :]

    in_maps = []
    for k in range(NCORES):
        m = dict(base)
        for p, (x, *_rest) in graphs.items():
            m[p + "xT"] = _prep_x(np.asarray(x), k)
            m.update({p + n: preps[p][k][n] for n in ("idxQ", "pos", "wv")})
        in_maps.append(m)

    res = run_bass_kernel_spmd(nc, in_maps, core_ids=list(range(NCORES)),
                               **(_run_opts or {}))
    emb1 = np.concatenate([res.results[k]["dout"] for k in range(NCORES)], axis=0)
    emb2 = np.concatenate([res.results[k]["sout"] for k in range(NCORES)], axis=0)
    if _run_opts:
        kernel.last_results = res
    return emb1, emb2
